# revision 10
# baseline (speedup 1.0000x reference)
"""Trainium2 Bass kernel for nn_AssembledBlock_6167573037591.

Mixture-of-expert CNN block: control net (GAP -> 1x1 -> relu -> 1x1 ->
softmax over 4 experts) produces per-(sample, out-channel) expert mixing
coefficients; three chained 3x3 convs (64->64 ch, 128x128 img, pad 1) run
with the per-sample mixed weights.

Distribution: pure data parallel over batch. B=16 samples over 8 cores ->
2 samples per core, full weights everywhere, no collectives.

Per-core plan (v3 - short prelude):
  - Conv core unchanged from v2: images padded 130x130 in SBUF (bf16),
    conv = 9 accumulating matmuls/tap into PSUM with the PE split into
    four concurrent 64x64 quadrant tiles; steady state measured ~98% of
    the bf16 PE roofline. The entire remaining cost was prelude (input
    DMA + control + weight transposes, ~85us) and output drain.
  - Inputs arrive bf16 (host-cast): x is 4MB instead of 8, expert
    weights 0.9MB, so the input DMA floor drops to ~13us. cw1/cw2 are
    host-transposed (no PE transposes / identity matrix needed), biases
    host-transposed+duplicated.
  - Mixed weights are produced DIRECTLY in conv layout via the DMA
    crossbar transpose: mixing runs per-sample on partitions 0:64 in a
    padded (tap, c-pad-128) layout, then ONE dma_start_transpose per
    (layer, sample) emits lw[128, (tap, oc)] - s1's block lands on
    partitions 64:128 for free (no staging copies, no partition-shift
    DMAs, no PSUM, no ACT backlog).
  - Pooling is split ACT (copy+accum, chunks 0-3) / DVE (copy + reduce,
    chunks 4-7) so it hides entirely under the input DMA.
  - Output is written bf16 (host upcast), one combined DMA per
    (row-group, sample) on sync/gpsimd only, keeping ACT under its
    per-row-group evacuation budget in layer 3.
"""

import os
import numpy as np

_STATE = {}

N_CORES = 8
S = 2            # samples per core
C = 64           # channels (in == out)
H = W = 128
HP = WP = 130    # padded
NP = HP * WP
E = 4            # experts
HID = 16
TEMP = 30.0
RPG = 8          # output rows per row-group (two 4-row halves via col tiles)
RG = H // RPG    # 16 row groups
NCHUNK = 8       # input DMA / pooling chunks (16 rows each)
T9 = 9           # conv taps


def _emit(tc, nc, dram, mybir, bass):
    from contextlib import ExitStack

    F32 = mybir.dt.float32
    BF16 = mybir.dt.bfloat16
    AF = mybir.ActivationFunctionType
    ALU = mybir.AluOpType

    xd, cw1td, cw2td, rwd, btd, od = dram

    ctx = ExitStack()
    with ctx:
        big = ctx.enter_context(tc.tile_pool(name="big", bufs=1))
        wpool = ctx.enter_context(tc.tile_pool(name="wts", bufs=1))
        small = ctx.enter_context(tc.tile_pool(name="small", bufs=1))
        mixs = ctx.enter_context(tc.tile_pool(name="mixs", bufs=4))
        stgx = ctx.enter_context(tc.tile_pool(name="stgx", bufs=8))
        wstgp = ctx.enter_context(tc.tile_pool(name="wstg", bufs=4))
        ostgp = ctx.enter_context(tc.tile_pool(name="ostg", bufs=4))
        cpsum = ctx.enter_context(tc.tile_pool(name="cpsum", bufs=8, space="PSUM"))

        # ---- persistent big buffers ----
        xpad = big.tile([128, NP], BF16)  # conv1 input / conv2 output
        ypad = big.tile([128, NP], BF16)  # conv1 output / conv2 input
        xv = xpad[:].rearrange("p (h w) -> p h w", h=HP, w=WP)
        yv = ypad[:].rearrange("p (h w) -> p h w", h=HP, w=WP)

        # raw expert weights, free layout (e, c, tap), partitions = oc
        rwsb = [wpool.tile([C, E * C * T9], BF16, name=f"rw{l}") for l in range(3)]
        # mixed weights per layer: cols 0:576 = s0 (parts 0:64 valid),
        # cols 576:1152 = s1 (parts 64:128 valid); free (tap, oc)
        lw = [wpool.tile([128, 2 * T9 * C], BF16, name=f"lw{l}") for l in range(3)]
        # final mixed agg per (layer, sample): [oc, (tap, cpad128)] bf16
        agg = [[wpool.tile([C, T9 * 128], BF16, name=f"agg{l}_{s}")
                for s in range(S)] for l in range(3)]

        cw1sb = small.tile([128, HID], F32)
        cw2sb = small.tile([HID, E * C], F32)
        btsb = [small.tile([128, E], F32, name=f"bt{l}") for l in range(3)]
        bmix = [small.tile([128, 1], F32, name=f"bmix{l}") for l in range(3)]
        bmix_sw = [small.tile([128, 1], F32, name=f"bmixsw{l}") for l in range(3)]
        psums = small.tile([128, NCHUNK], F32)   # pooling partials
        pooled = small.tile([128, 1], F32)
        pooleds = small.tile([128, 1], F32)      # scaled by 1/(H*W)
        hid_sb = small.tile([HID, S], F32)
        exp_sb = small.tile([C, S * E], F32)     # col = 2*e + s
        ssum = small.tile([C, S], F32)
        srec = small.tile([C, S], F32)
        coeff = small.tile([C, S * E], F32)      # col = 2*e + s
        coeff2 = small.tile([128, E], F32)       # partition = 64*s + oc
        css1 = small.tile([C, E], F32)           # staging for coeff2 upper

        # zero halo borders of both big buffers
        for v in (xv, yv):
            nc.vector.memset(v[:, 0, :], 0.0)
            nc.vector.memset(v[:, HP - 1, :], 0.0)
            nc.vector.memset(v[:, 1:HP - 1, 0:1], 0.0)
            nc.vector.memset(v[:, 1:HP - 1, WP - 1:WP], 0.0)

        # ---- input DMA: 8 chunks x [128, 16*128] bf16 on sync+gpsimd ----
        xd128 = xd.ap().rearrange("s c h w -> (s c) h w")
        rows_per_chunk = H // NCHUNK
        xstgs = []
        for rb in range(NCHUNK):
            r0 = rb * rows_per_chunk
            xstg = stgx.tile([128, rows_per_chunk * W], BF16, name="xstg",
                             tag="xstg")
            xsv = xstg[:].rearrange("p (h w) -> p h w", h=rows_per_chunk, w=W)
            eng = nc.sync if rb % 2 == 0 else nc.gpsimd
            eng.dma_start(out=xsv[:, :, :], in_=xd128[:, r0:r0 + rows_per_chunk, :])
            xstgs.append((r0, xstg, xsv))

        # ---- constants / static loads (gpsimd SWDGE queue) ----
        nc.gpsimd.dma_start(out=cw1sb[:], in_=cw1td.ap())
        nc.gpsimd.dma_start(out=cw2sb[:], in_=cw2td.ap())
        for l in range(3):
            nc.gpsimd.dma_start(out=btsb[l][:], in_=btd[l].ap())
        for l in range(3):
            # [E, C, 576] -> partitions oc, free (e, c, tap)
            nc.gpsimd.dma_start(
                out=rwsb[l][:].rearrange("o (e f) -> o e f", e=E),
                in_=rwd[l].ap().rearrange("e o f -> o e f"),
            )

        # ---- pooling: ACT copy+accum (chunks 0-3), DVE copy+reduce (4-7)
        for rb, (r0, xstg, xsv) in enumerate(xstgs):
            dst = xv[:, 1 + r0:1 + r0 + rows_per_chunk, 1:1 + W]
            if rb not in (4, 5):
                nc.scalar.activation(dst, xsv[:, :, :], AF.Copy,
                                     accum_out=psums[:, rb:rb + 1])
            else:
                nc.vector.tensor_copy(dst, xsv[:, :, :])
                nc.vector.tensor_reduce(
                    out=psums[:, rb:rb + 1], in_=xstg[:],
                    axis=mybir.AxisListType.X, op=ALU.add,
                )

        nc.vector.tensor_reduce(
            out=pooled[:], in_=psums[:], axis=mybir.AxisListType.X, op=ALU.add
        )
        nc.scalar.mul(pooleds[:], pooled[:], 1.0 / (H * W))

        # ---- control module ----
        hid_ps = cpsum.tile([HID, S], F32, tag="conv")
        nc.tensor.matmul(hid_ps[:, 0:1], cw1sb[0:64, :], pooleds[0:64, :],
                         tile_position=(0, 0), start=True, stop=True)
        nc.tensor.matmul(hid_ps[:, 1:2], cw1sb[64:128, :], pooleds[64:128, :],
                         tile_position=(64, 0), start=True, stop=True,
                         skip_group_check=True)
        nc.scalar.activation(hid_sb[:], hid_ps[:], AF.Relu)
        # logits for all experts into one PSUM tile [C, (e, s)]
        lg = cpsum.tile([C, S * E], F32, tag="conv")
        cw2v = cw2sb[:].rearrange("p (o e) -> p o e", e=E)
        for e in range(E):
            nc.tensor.matmul(lg[:, S * e:S * (e + 1)], cw2v[:, :, e], hid_sb[:],
                             start=True, stop=True, skip_group_check=(e > 0))
        nc.scalar.activation(exp_sb[:], lg[:], AF.Exp, scale=1.0 / TEMP)
        ev = exp_sb[:].rearrange("p (e s) -> p e s", s=S)
        cv = coeff[:].rearrange("p (e s) -> p e s", s=S)
        for s in range(S):
            nc.vector.tensor_reduce(
                out=ssum[:, s:s + 1], in_=ev[:, :, s],
                axis=mybir.AxisListType.X, op=ALU.add,
            )
        nc.vector.reciprocal(srec[:], ssum[:])
        for s in range(S):
            nc.vector.tensor_scalar_mul(cv[:, :, s], ev[:, :, s], srec[:, s:s + 1])

        # coeff2 [p=(s,oc), e] for bias mixing (off critical path)
        nc.vector.tensor_copy(coeff2[0:64, :], cv[:, :, 0])
        nc.vector.tensor_copy(css1[:], cv[:, :, 1])
        nc.sync.dma_start(out=coeff2[64:128, :], in_=css1[:])

        # ---- mix expert weights per (layer, sample) ----
        # s0: DVE scalar_tensor_tensor chain (per-partition ptr scalars).
        # s1: ptr-scalar tensor ops are illegal on Pool, so ACT does the
        #     4 scaled products (activation Copy, scale=coeff) and Pool
        #     does the 3-add tree.
        def emit_mix(l, s):
            cs = [coeff[:, S * e + s:S * e + s + 1] for e in range(E)]
            rwv = [
                rwsb[l][:, 576 * e:576 * (e + 1)].rearrange(
                    "p (c t) -> p t c", t=T9)
                for e in range(E)
            ]
            # final lands in padded (t, cpad128) layout; s1 data sits at
            # cpad 64:128 so its transpose hits partitions 64:128.
            c0 = 64 * s
            av = agg[l][s][:].rearrange(
                "p (t k) -> p t k", k=128)[:, :, c0:c0 + 64]
            if s == 0:
                t1 = mixs.tile([C, C * T9], F32, name="mx1", tag="mx0")
                t2 = mixs.tile([C, C * T9], F32, name="mx2", tag="mx0")
                v1 = t1[:].rearrange("p (t c) -> p t c", t=T9)
                v2 = t2[:].rearrange("p (t c) -> p t c", t=T9)
                nc.vector.tensor_scalar_mul(v1, rwv[0], cs[0])
                nc.vector.scalar_tensor_tensor(
                    out=v2, in0=rwv[1], scalar=cs[1], in1=v1,
                    op0=ALU.mult, op1=ALU.add)
                nc.vector.scalar_tensor_tensor(
                    out=v1, in0=rwv[2], scalar=cs[2], in1=v2,
                    op0=ALU.mult, op1=ALU.add)
                nc.vector.scalar_tensor_tensor(
                    out=av, in0=rwv[3], scalar=cs[3], in1=v1,
                    op0=ALU.mult, op1=ALU.add)
            else:
                ps = [mixs.tile([C, C * T9], F32, name=f"mp{e}", tag="mxp")
                      for e in range(E)]
                qs = [mixs.tile([C, C * T9], F32, name=f"mq{i}", tag="mxq")
                      for i in range(2)]
                for e in range(E):
                    nc.scalar.activation(
                        ps[e][:].rearrange("p (t c) -> p t c", t=T9),
                        rwv[e], AF.Copy, scale=cs[e])
                nc.gpsimd.tensor_tensor(qs[0][:], ps[0][:], ps[1][:],
                                        op=ALU.add)
                nc.gpsimd.tensor_tensor(qs[1][:], ps[2][:], ps[3][:],
                                        op=ALU.add)
                nc.gpsimd.tensor_tensor(
                    av,
                    qs[0][:].rearrange("p (t c) -> p t c", t=T9),
                    qs[1][:].rearrange("p (t c) -> p t c", t=T9),
                    op=ALU.add)

        def emit_transpose(l):
            for s in range(S):
                eng = nc.sync if s == 0 else nc.scalar
                eng.dma_start_transpose(
                    out=lw[l][:, 576 * s:576 * (s + 1)].rearrange(
                        "p (t o) -> p t o", o=C),
                    in_=agg[l][s][:],
                )

        def emit_bias(l):
            bt = mixs.tile([128, E], F32, name="btm", tag="btm")
            nc.vector.tensor_tensor(bt[:], btsb[l][:], coeff2[:], op=ALU.mult)
            nc.vector.tensor_reduce(
                out=bmix[l][:], in_=bt[:], axis=mybir.AxisListType.X, op=ALU.add
            )
            nc.sync.dma_start(out=bmix_sw[l][64:128, :], in_=bmix[l][0:64, :])
            nc.sync.dma_start(out=bmix_sw[l][0:64, :], in_=bmix[l][64:128, :])

        # layer 0 first (critical path), then biases, then layers 1-2
        emit_mix(0, 0)
        emit_mix(0, 1)
        emit_transpose(0)
        emit_bias(0)
        for l in (1, 2):
            emit_mix(l, 0)
            emit_mix(l, 1)
            emit_transpose(l)
            emit_bias(l)

        def emit_conv(l):
            srcv = (xv, yv, xv)[l]
            dstv = (yv, xv, None)[l]
            lw0 = lw[l][0:64, 0:576].rearrange("p (t b) -> p t b", b=C)
            lw1 = lw[l][64:128, 576:1152].rearrange("p (t b) -> p t b", b=C)
            for rg in range(RG):
                r0 = RPG * rg
                psA = cpsum.tile([128, 4 * W], F32, tag="conv")
                psB = cpsum.tile([128, 4 * W], F32, tag="conv")
                for t in range(T9):
                    dy, dx = divmod(t, 3)
                    st, sp = (t == 0), (t == 8)
                    rhs0 = srcv[0:64, r0 + dy:r0 + dy + 4, dx:dx + W]
                    rhs1 = srcv[0:64, r0 + 4 + dy:r0 + 4 + dy + 4, dx:dx + W]
                    rhs2 = srcv[64:128, r0 + dy:r0 + dy + 4, dx:dx + W]
                    rhs3 = srcv[64:128, r0 + 4 + dy:r0 + 4 + dy + 4, dx:dx + W]
                    nc.tensor.matmul(
                        psA[0:64, :], lw0[:, t, :], rhs0,
                        tile_position=(0, 0), start=st, stop=sp,
                        skip_group_check=True,
                    )
                    nc.tensor.matmul(
                        psA[64:128, :], lw0[:, t, :], rhs1,
                        tile_position=(0, 64), start=st, stop=sp,
                        skip_group_check=True,
                    )
                    nc.tensor.matmul(
                        psB[0:64, :], lw1[:, t, :], rhs2,
                        tile_position=(64, 0), start=st, stop=sp,
                        skip_group_check=True,
                    )
                    nc.tensor.matmul(
                        psB[64:128, :], lw1[:, t, :], rhs3,
                        tile_position=(64, 64), start=st, stop=sp,
                        skip_group_check=True,
                    )
                psAv = psA[:].rearrange("p (h w) -> p h w", h=4, w=W)
                psBv = psB[:].rearrange("p (h w) -> p h w", h=4, w=W)
                if l < 2:
                    # home quarters: direct bias-add + cast into padded dst
                    nc.scalar.activation(
                        dstv[0:64, r0 + 1:r0 + 5, 1:1 + W], psAv[0:64],
                        AF.Identity, bias=bmix[l][0:64, 0:1],
                    )
                    nc.vector.tensor_scalar_add(
                        dstv[64:128, r0 + 5:r0 + 9, 1:1 + W], psBv[64:128],
                        bmix[l][64:128, 0:1],
                    )
                    # off-home quarters: stage + DMA partition shift
                    stA = wstgp.tile([128, 4 * W], BF16, tag="wstg")
                    nc.scalar.activation(
                        stA[64:128, :], psA[64:128, :],
                        AF.Identity, bias=bmix_sw[l][64:128, 0:1],
                    )
                    nc.vector.tensor_scalar_add(
                        stA[0:64, :], psB[0:64, :], bmix_sw[l][0:64, 0:1]
                    )
                    stAv = stA[:].rearrange("p (h w) -> p h w", h=4, w=W)
                    eng1 = nc.gpsimd if rg % 2 == 0 else nc.sync
                    eng1.dma_start(
                        out=dstv[0:64, r0 + 5:r0 + 9, 1:1 + W],
                        in_=stAv[64:128],
                    )
                    eng1.dma_start(
                        out=dstv[64:128, r0 + 1:r0 + 5, 1:1 + W],
                        in_=stAv[0:64],
                    )
                else:
                    # final layer: bias-add into bf16 staging, DMA to DRAM
                    ostA = ostgp.tile([128, 4 * W], BF16, tag="ostg")
                    ostB = ostgp.tile([128, 4 * W], BF16, tag="ostg")
                    nc.scalar.activation(
                        ostA[0:64, :], psA[0:64, :],
                        AF.Identity, bias=bmix[l][0:64, 0:1],
                    )
                    nc.scalar.activation(
                        ostA[64:128, :], psA[64:128, :],
                        AF.Identity, bias=bmix_sw[l][64:128, 0:1],
                    )
                    nc.vector.tensor_scalar_add(
                        ostB[0:64, :], psB[0:64, :], bmix_sw[l][0:64, 0:1]
                    )
                    nc.vector.tensor_scalar_add(
                        ostB[64:128, :], psB[64:128, :], bmix[l][64:128, 0:1]
                    )
                    # two DMAs per sample (one per 4-row half); sync and
                    # gpsimd only, keeping ACT under its evacuation budget
                    for s, ost in ((0, ostA), (1, ostB)):
                        ov = ost[:].rearrange("p (h w) -> p h w", h=4, w=W)
                        eng = nc.sync if s == 0 else nc.gpsimd
                        eng.dma_start(
                            out=od.ap()[s][:, r0:r0 + 4, :], in_=ov[0:64]
                        )
                        eng.dma_start(
                            out=od.ap()[s][:, r0 + 4:r0 + 8, :], in_=ov[64:128]
                        )

        for l in range(3):
            emit_conv(l)


def _get_nc():
    if "nc" in _STATE:
        return _STATE["nc"]
    import concourse.bass as bass
    import concourse.tile as tile
    from concourse import bacc, mybir

    F32 = mybir.dt.float32
    BF16 = mybir.dt.bfloat16
    nc = bacc.Bacc(
        "TRN2", target_bir_lowering=False, debug=False, num_devices=N_CORES
    )
    xd = nc.dram_tensor("x", [S, C, H, W], BF16, kind="ExternalInput")
    cw1td = nc.dram_tensor("cw1t", [128, HID], F32, kind="ExternalInput")
    cw2td = nc.dram_tensor("cw2t", [HID, E * C], F32, kind="ExternalInput")
    rwd = [
        nc.dram_tensor(f"rw{l+1}", [E, C, C * T9], BF16, kind="ExternalInput")
        for l in range(3)
    ]
    btd = [
        nc.dram_tensor(f"bt{l+1}", [128, E], F32, kind="ExternalInput")
        for l in range(3)
    ]
    od = nc.dram_tensor("out", [S, C, H, W], BF16, kind="ExternalOutput")

    with tile.TileContext(nc) as tc:
        _emit(tc, nc, (xd, cw1td, cw2td, rwd, btd, od), mybir, bass)
    nc.compile()
    _STATE["nc"] = nc
    return nc


def prep_in_maps(inputs):
    """Host-side marshalling: dtype casts + static layout transforms."""
    import ml_dtypes

    BF = ml_dtypes.bfloat16
    arr = {k: np.asarray(v, dtype=np.float32) for k, v in inputs.items()}
    x = np.ascontiguousarray(arr["x"].astype(BF))
    shared = {
        "cw1t": np.ascontiguousarray(
            np.concatenate([arr["cw1"].T, arr["cw1"].T], axis=0)),
        "cw2t": np.ascontiguousarray(arr["cw2"].T),
    }
    for l in range(3):
        w = arr[f"w{l+1}"]
        shared[f"rw{l+1}"] = np.ascontiguousarray(
            w.reshape(E, C, C * T9).astype(BF))
        b = arr[f"b{l+1}"].T
        shared[f"bt{l+1}"] = np.ascontiguousarray(
            np.concatenate([b, b], axis=0))
    return [
        {"x": np.ascontiguousarray(x[S * i:S * (i + 1)]), **shared}
        for i in range(N_CORES)
    ]


def kernel(**inputs):
    from concourse.bass_utils import run_bass_kernel_spmd

    nc = _get_nc()
    in_maps = prep_in_maps(inputs)
    trace = bool(int(os.environ.get("KBENCH_TRACE", "0")))
    last_err = None
    for _attempt in range(3):
        try:
            res = run_bass_kernel_spmd(
                nc, in_maps, list(range(N_CORES)), trace=trace
            )
            _STATE["last"] = res
            return np.concatenate(
                [np.asarray(res.results[i]["out"]).astype(np.float32)
                 for i in range(N_CORES)], axis=0
            )
        except Exception as e:  # transient NRT device faults observed
            last_err = e
            if "UNRECOVERABLE" not in str(e) and "UNAVAILABLE" not in str(e):
                raise
    raise last_err


# revision 28
# speedup vs baseline: 1.3523x; 1.3523x over previous
"""Trainium2 Bass kernel for nn_AssembledBlock_6167573037591.

Mixture-of-expert CNN block: control net (GAP -> 1x1 -> relu -> 1x1 ->
softmax over 4 experts) produces per-(sample, out-channel) expert mixing
coefficients; three chained 3x3 convs (64->64 ch, 128x128 img, pad 1) run
with the per-sample mixed weights.

Distribution: pure data parallel over batch. B=16 samples over 8 cores ->
2 samples per core, full weights everywhere, no collectives.

Per-core plan (v4 - short prelude):
  - Conv core unchanged from v2: images padded 130x130 in SBUF (bf16),
    conv = 9 accumulating matmuls/tap into PSUM with the PE split into
    four concurrent 64x64 quadrant tiles; steady state measured ~98% of
    the bf16 PE roofline. The entire remaining cost was prelude (input
    DMA + control + weight transposes, ~85us) and output drain.
  - Inputs arrive bf16 (host-cast): x is 4MB instead of 8, expert
    weights 0.9MB, so the input DMA floor drops to ~13us. cw1/cw2 are
    host-transposed (no PE transposes / identity matrix needed), biases
    host-transposed+duplicated.
  - Mixed weights are produced DIRECTLY in conv layout via the DMA
    crossbar transpose: mixing runs per-sample on partitions 0:64 in a
    padded (tap, c-pad-128) layout, then ONE dma_start_transpose per
    (layer, sample) emits lw[128, (tap, oc)] - s1's block lands on
    partitions 64:128 for free (no staging copies, no partition-shift
    DMAs, no PSUM, no ACT backlog).
  - Pooling is split ACT (copy+accum, chunks 0-3) / DVE (copy + reduce,
    chunks 4-7) so it hides entirely under the input DMA.
  - Output is written bf16 (host upcast), one combined DMA per
    (row-group, sample) on sync/gpsimd only, keeping ACT under its
    per-row-group evacuation budget in layer 3.
"""

import os
import numpy as np

_STATE = {}

N_CORES = 8
S = 2            # samples per core
C = 64           # channels (in == out)
H = W = 128
HP = WP = 130    # padded
NP = HP * WP
E = 4            # experts
HID = 16
TEMP = 30.0
RPG = 8          # output rows per row-group (two 4-row halves via col tiles)
RG = H // RPG    # 16 row groups
NCHUNK = 8       # input DMA / pooling chunks (16 rows each)
T9 = 9           # conv taps


def _emit(tc, nc, dram, mybir, bass):
    from contextlib import ExitStack

    F32 = mybir.dt.float32
    BF16 = mybir.dt.bfloat16
    AF = mybir.ActivationFunctionType
    ALU = mybir.AluOpType

    xd, cw1td, cw2td, rwd, btd, od, identd = dram

    ctx = ExitStack()
    with ctx:
        big = ctx.enter_context(tc.tile_pool(name="big", bufs=1))
        wpool = ctx.enter_context(tc.tile_pool(name="wts", bufs=1))
        small = ctx.enter_context(tc.tile_pool(name="small", bufs=1))
        mixs = ctx.enter_context(tc.tile_pool(name="mixs", bufs=4))
        stgx = ctx.enter_context(tc.tile_pool(name="stgx", bufs=8))
        wstgp = ctx.enter_context(tc.tile_pool(name="wstg", bufs=8))
        ostgp = ctx.enter_context(tc.tile_pool(name="ostg", bufs=8))
        cpsum = ctx.enter_context(tc.tile_pool(name="cpsum", bufs=6, space="PSUM"))
        tpsum = ctx.enter_context(tc.tile_pool(name="tpsum", bufs=2, space="PSUM"))

        # ---- persistent big buffers ----
        xpad = big.tile([128, NP], BF16)  # conv1 input / conv2 output
        ypad = big.tile([128, NP], BF16)  # conv1 output / conv2 input
        xv = xpad[:].rearrange("p (h w) -> p h w", h=HP, w=WP)
        yv = ypad[:].rearrange("p (h w) -> p h w", h=HP, w=WP)

        # raw expert weights, partitions (s,oc) duplicated halves, free
        # (e, tap, c)
        rwsb = [wpool.tile([128, E * T9 * C], BF16, name=f"rw{l}")
                for l in range(3)]
        # mixed weights per layer: cols 0:576 = s0 (parts 0:64 valid),
        # cols 576:1152 = s1 (parts 64:128 valid); free (tap, oc)
        lw = [wpool.tile([128, 2 * T9 * C], BF16, name=f"lw{l}") for l in range(3)]
        # mixed agg per layer, both samples: [(s,oc), (tap, c)] bf16
        agg2 = [wpool.tile([128, T9 * C], BF16, name=f"agg{l}")
                for l in range(3)]
        mxf = [wpool.tile([128, T9 * C], F32, name=f"mxf{i}")
               for i in range(2)]
        stg = [wpool.tile([C, T9 * C], BF16, name=f"stg{l}")
               for l in range(3)]
        identsb = small.tile([128, 128], BF16)

        cw1sb = small.tile([128, 2 * HID], F32)
        cw2sb = small.tile([2 * HID, E * 128], BF16)
        btsb = [small.tile([128, E], F32, name=f"bt{l}") for l in range(3)]
        bmix = [small.tile([128, 1], F32, name=f"bmix{l}") for l in range(3)]
        bmix_sw = [small.tile([128, 1], F32, name=f"bmixsw{l}") for l in range(3)]
        psums = small.tile([128, 2 * NCHUNK], F32)  # pooling partials
        pooled = small.tile([128, 1], F32)
        hid_sb = small.tile([2 * HID, 1], BF16)
        ssum = small.tile([128, 1], F32)
        srec = small.tile([128, 1], F32)
        srec_sw = small.tile([128, 1], F32)

        # zero halo borders of both big buffers
        for v in (xv, yv):
            nc.vector.memset(v[:, 0, :], 0.0)
            nc.vector.memset(v[:, HP - 1, :], 0.0)
            nc.vector.memset(v[:, 1:HP - 1, 0:1], 0.0)
            nc.vector.memset(v[:, 1:HP - 1, WP - 1:WP], 0.0)

        # ---- input DMA: 8 chunks x [128, 16*128] bf16 on sync+gpsimd ----
        xd128 = xd.ap().rearrange("s c h w -> (s c) h w")
        rows_per_chunk = H // NCHUNK
        xstgs = []
        for rb in range(NCHUNK):
            r0 = rb * rows_per_chunk
            xstg = stgx.tile([128, rows_per_chunk * W], BF16, name="xstg",
                             tag="xstg")
            xsv = xstg[:].rearrange("p (h w) -> p h w", h=rows_per_chunk, w=W)
            eng = nc.sync if rb % 2 == 0 else nc.gpsimd
            eng.dma_start(out=xsv[:, :, :], in_=xd128[:, r0:r0 + rows_per_chunk, :])
            xstgs.append((r0, xstg, xsv))

        # ---- constants / static loads (gpsimd SWDGE queue) ----
        nc.gpsimd.dma_start(out=identsb[:], in_=identd.ap())
        nc.gpsimd.dma_start(out=cw1sb[:], in_=cw1td.ap())
        nc.gpsimd.dma_start(out=cw2sb[:], in_=cw2td.ap())
        for l in range(3):
            nc.gpsimd.dma_start(out=btsb[l][:], in_=btd[l].ap())
        for l in range(3):
            # [E, 128, 576] -> partitions (s,oc), free (e, tap, c)
            nc.gpsimd.dma_start(
                out=rwsb[l][:].rearrange("o (e f) -> o e f", e=E),
                in_=rwd[l].ap().rearrange("e o f -> o e f"),
            )

        # ---- pooling: per chunk, ACT fused copy+accum on rows 0:10 and
        # DVE copy + reduce on rows 10:16 (balances both engine pipes)
        RS = 10
        for rb, (r0, xstg, xsv) in enumerate(xstgs):
            nc.scalar.activation(
                xv[:, 1 + r0:1 + r0 + RS, 1:1 + W], xsv[:, 0:RS, :],
                AF.Copy, accum_out=psums[:, 2 * rb:2 * rb + 1])
            nc.vector.tensor_copy(
                xv[:, 1 + r0 + RS:1 + r0 + rows_per_chunk, 1:1 + W],
                xsv[:, RS:rows_per_chunk, :])
            nc.vector.tensor_reduce(
                out=psums[:, 2 * rb + 1:2 * rb + 2],
                in_=xstg[:, RS * W:rows_per_chunk * W],
                axis=mybir.AxisListType.X, op=ALU.add,
            )

        nc.vector.tensor_reduce(
            out=pooled[:], in_=psums[:], axis=mybir.AxisListType.X, op=ALU.add
        )

        # ---- control module (block-diagonal: coeff2 emitted directly in
        # the (s,oc)-partition layout; no partition shifts) ----
        hid_ps = cpsum.tile([2 * HID, 1], F32, tag="conv")
        nc.tensor.matmul(hid_ps[:], cw1sb[:], pooled[:],
                         start=True, stop=True)
        nc.scalar.activation(hid_sb[:], hid_ps[:], AF.Relu)
        lg = cpsum.tile([128, E], F32, tag="conv")
        for e in range(E):
            nc.tensor.matmul(lg[:, e:e + 1], cw2sb[:, 128 * e:128 * (e + 1)],
                             hid_sb[:], start=True, stop=True,
                             skip_group_check=(e > 0))
        exp2 = small.tile([128, E], F32)
        nc.scalar.activation(exp2[:], lg[:], AF.Exp, scale=1.0 / TEMP)

        def emit_softmax_denom():
            # off the mix critical path: 1/Z folds into PSUM evacuation
            nc.vector.tensor_reduce(
                out=ssum[:], in_=exp2[:], axis=mybir.AxisListType.X,
                op=ALU.add)
            nc.vector.reciprocal(srec[:], ssum[:])
            nc.sync.dma_start(out=srec_sw[64:128, :], in_=srec[0:64, :])
            nc.sync.dma_start(out=srec_sw[0:64, :], in_=srec[64:128, :])

        # ---- mix expert weights: unit-stride DVE chains per tap-group,
        # both samples at once ([128, cols]; per-partition UNNORMALIZED
        # exp scalars - softmax 1/Z is applied at PSUM evacuation).
        def emit_mix(l, c0, c1):
            cs = [exp2[:, e:e + 1] for e in range(E)]
            rwv = [rwsb[l][:, 576 * e + c0:576 * e + c1] for e in range(E)]
            w = c1 - c0
            nc.vector.tensor_scalar_mul(mxf[0][:, 0:w], rwv[0], cs[0])
            nc.vector.scalar_tensor_tensor(
                out=mxf[1][:, 0:w], in0=rwv[1], scalar=cs[1],
                in1=mxf[0][:, 0:w], op0=ALU.mult, op1=ALU.add)
            nc.vector.scalar_tensor_tensor(
                out=mxf[0][:, 0:w], in0=rwv[2], scalar=cs[2],
                in1=mxf[1][:, 0:w], op0=ALU.mult, op1=ALU.add)
            nc.vector.scalar_tensor_tensor(
                out=agg2[l][:, c0:c1], in0=rwv[3], scalar=cs[3],
                in1=mxf[0][:, 0:w], op0=ALU.mult, op1=ALU.add)

        def emit_transpose(l, groups=((0, 4), (4, 8), (8, 9)), shift=True):
            # PE transposes agg2 (s,o)x(c) per tap -> [c, (s,o)]; 4 taps
            # batched per PSUM bank, then one strided copy per half (both
            # on ACT, keeping DVE free for the next mix group) and a
            # single partition-shift DMA for the s1 block.
            for g0, g1 in groups:
                ng = g1 - g0
                tp = tpsum.tile([C, 128 * ng], BF16, tag="tp")
                for t in range(g0, g1):
                    nc.tensor.matmul(
                        tp[:, 128 * (t - g0):128 * (t - g0 + 1)],
                        agg2[l][:, C * t:C * (t + 1)], identsb[:],
                        is_transpose=True, start=True, stop=True,
                        skip_group_check=True,
                    )
                tpv = tp[:].rearrange("p (g k) -> p g k", k=128)
                nc.scalar.activation(
                    lw[l][0:64, C * g0:C * g1].rearrange(
                        "p (g o) -> p g o", o=C),
                    tpv[:, :, 0:64], AF.Copy,
                )
                nc.scalar.activation(
                    stg[l][:, C * g0:C * g1].rearrange(
                        "p (g o) -> p g o", o=C),
                    tpv[:, :, 64:128], AF.Copy,
                )
            if shift:
                nc.sync.dma_start(out=lw[l][64:128, 576:1152], in_=stg[l][:])

        def emit_bias(l):
            bt = mixs.tile([128, E], F32, name="btm", tag="btm")
            btu = mixs.tile([128, 1], F32, name="btu", tag="btm")
            nc.vector.tensor_tensor(bt[:], btsb[l][:], exp2[:], op=ALU.mult)
            nc.vector.tensor_reduce(
                out=btu[:], in_=bt[:], axis=mybir.AxisListType.X, op=ALU.add
            )
            nc.vector.tensor_scalar_mul(bmix[l][:], btu[:], srec[:])
            nc.sync.dma_start(out=bmix_sw[l][64:128, :], in_=bmix[l][0:64, :])
            nc.sync.dma_start(out=bmix_sw[l][0:64, :], in_=bmix[l][64:128, :])

        # layer 0 first (critical path; tap-group pipelined so transposes
        # begin after the first 4 taps are mixed); layers 1-2 follow,
        # their PE transposes slotted between conv0 row-groups
        emit_mix(0, 0, 4 * C)
        emit_transpose(0, groups=((0, 4),), shift=False)
        emit_mix(0, 4 * C, 9 * C)
        emit_transpose(0, groups=((4, 8), (8, 9)))
        emit_softmax_denom()
        emit_bias(0)
        emit_mix(1, 0, 9 * C)
        emit_bias(1)
        emit_mix(2, 0, 9 * C)
        emit_bias(2)

        def emit_conv(l, rgs=None):
            srcv = (xv, yv, xv)[l]
            dstv = (yv, xv, None)[l]
            lw0 = lw[l][0:64, 0:576].rearrange("p (t b) -> p t b", b=C)
            lw1 = lw[l][64:128, 576:1152].rearrange("p (t b) -> p t b", b=C)
            for rg in (range(RG) if rgs is None else rgs):
                r0 = RPG * rg
                psA = cpsum.tile([128, 4 * W], F32, tag="conv")
                psB = cpsum.tile([128, 4 * W], F32, tag="conv")
                for t in range(T9):
                    dy, dx = divmod(t, 3)
                    st, sp = (t == 0), (t == 8)
                    rhs0 = srcv[0:64, r0 + dy:r0 + dy + 4, dx:dx + W]
                    rhs1 = srcv[0:64, r0 + 4 + dy:r0 + 4 + dy + 4, dx:dx + W]
                    rhs2 = srcv[64:128, r0 + dy:r0 + dy + 4, dx:dx + W]
                    rhs3 = srcv[64:128, r0 + 4 + dy:r0 + 4 + dy + 4, dx:dx + W]
                    nc.tensor.matmul(
                        psA[0:64, :], lw0[:, t, :], rhs0,
                        tile_position=(0, 0), start=st, stop=sp,
                        skip_group_check=True,
                    )
                    nc.tensor.matmul(
                        psA[64:128, :], lw0[:, t, :], rhs1,
                        tile_position=(0, 64), start=st, stop=sp,
                        skip_group_check=True,
                    )
                    nc.tensor.matmul(
                        psB[0:64, :], lw1[:, t, :], rhs2,
                        tile_position=(64, 0), start=st, stop=sp,
                        skip_group_check=True,
                    )
                    nc.tensor.matmul(
                        psB[64:128, :], lw1[:, t, :], rhs3,
                        tile_position=(64, 64), start=st, stop=sp,
                        skip_group_check=True,
                    )
                psAv = psA[:].rearrange("p (h w) -> p h w", h=4, w=W)
                psBv = psB[:].rearrange("p (h w) -> p h w", h=4, w=W)
                if l < 2:
                    # home quarters: 1/Z scale + bias + cast into padded dst
                    nc.scalar.activation(
                        dstv[0:64, r0 + 1:r0 + 5, 1:1 + W], psAv[0:64],
                        AF.Identity, bias=bmix[l][0:64, 0:1],
                        scale=srec[0:64, 0:1],
                    )
                    nc.vector.tensor_scalar(
                        dstv[64:128, r0 + 5:r0 + 9, 1:1 + W], psBv[64:128],
                        srec[64:128, 0:1], bmix[l][64:128, 0:1],
                        op0=ALU.mult, op1=ALU.add,
                    )
                    # off-home quarters: stage + DMA partition shift
                    stA = wstgp.tile([128, 4 * W], BF16, tag="wstg")
                    nc.scalar.activation(
                        stA[64:128, :], psA[64:128, :],
                        AF.Identity, bias=bmix_sw[l][64:128, 0:1],
                        scale=srec_sw[64:128, 0:1],
                    )
                    nc.vector.tensor_scalar(
                        stA[0:64, :], psB[0:64, :],
                        srec_sw[0:64, 0:1], bmix_sw[l][0:64, 0:1],
                        op0=ALU.mult, op1=ALU.add,
                    )
                    stAv = stA[:].rearrange("p (h w) -> p h w", h=4, w=W)
                    nc.gpsimd.dma_start(
                        out=dstv[0:64, r0 + 5:r0 + 9, 1:1 + W],
                        in_=stAv[64:128],
                    )
                    nc.sync.dma_start(
                        out=dstv[64:128, r0 + 1:r0 + 5, 1:1 + W],
                        in_=stAv[0:64],
                    )
                else:
                    # final layer: bias-add into bf16 staging, DMA to DRAM
                    ostA = ostgp.tile([128, 4 * W], BF16, tag="ostg")
                    ostB = ostgp.tile([128, 4 * W], BF16, tag="ostg")
                    nc.scalar.activation(
                        ostA[0:64, :], psA[0:64, :],
                        AF.Identity, bias=bmix[l][0:64, 0:1],
                        scale=srec[0:64, 0:1],
                    )
                    nc.scalar.activation(
                        ostA[64:128, :], psA[64:128, :],
                        AF.Identity, bias=bmix_sw[l][64:128, 0:1],
                        scale=srec_sw[64:128, 0:1],
                    )
                    nc.vector.tensor_scalar(
                        ostB[0:64, :], psB[0:64, :],
                        srec_sw[0:64, 0:1], bmix_sw[l][0:64, 0:1],
                        op0=ALU.mult, op1=ALU.add,
                    )
                    nc.vector.tensor_scalar(
                        ostB[64:128, :], psB[64:128, :],
                        srec[64:128, 0:1], bmix[l][64:128, 0:1],
                        op0=ALU.mult, op1=ALU.add,
                    )
                    # two DMAs per sample (one per 4-row half); sync takes
                    # 3 of 4 (cheap issues), gpsimd 1, ACT stays evac-only
                    engs4 = ((nc.sync, nc.scalar, nc.gpsimd, nc.sync)
                             if rg >= 13 else
                             (nc.sync, nc.sync, nc.gpsimd, nc.sync))
                    for s, ost in ((0, ostA), (1, ostB)):
                        ov = ost[:].rearrange("p (h w) -> p h w", h=4, w=W)
                        engs4[2 * s].dma_start(
                            out=od.ap()[s][:, r0:r0 + 4, :], in_=ov[0:64]
                        )
                        engs4[2 * s + 1].dma_start(
                            out=od.ap()[s][:, r0 + 4:r0 + 8, :], in_=ov[64:128]
                        )

        emit_conv(0, range(0, 2))
        emit_transpose(1)
        emit_conv(0, range(2, 4))
        emit_transpose(2)
        emit_conv(0, range(4, RG))
        emit_conv(1)
        emit_conv(2)


def _get_nc():
    if "nc" in _STATE:
        return _STATE["nc"]
    import concourse.bass as bass
    import concourse.tile as tile
    from concourse import bacc, mybir

    F32 = mybir.dt.float32
    BF16 = mybir.dt.bfloat16
    nc = bacc.Bacc(
        "TRN2", target_bir_lowering=False, debug=False, num_devices=N_CORES
    )
    xd = nc.dram_tensor("x", [S, C, H, W], BF16, kind="ExternalInput")
    cw1td = nc.dram_tensor("cw1t", [128, 2 * HID], F32, kind="ExternalInput")
    cw2td = nc.dram_tensor("cw2t", [2 * HID, E * 128], BF16,
                           kind="ExternalInput")
    rwd = [
        nc.dram_tensor(f"rw{l+1}", [E, 128, T9 * C], BF16,
                       kind="ExternalInput")
        for l in range(3)
    ]
    btd = [
        nc.dram_tensor(f"bt{l+1}", [128, E], F32, kind="ExternalInput")
        for l in range(3)
    ]
    od = nc.dram_tensor("out", [S, C, H, W], BF16, kind="ExternalOutput")
    identd = nc.dram_tensor("ident", [128, 128], BF16, kind="ExternalInput")

    with tile.TileContext(nc) as tc:
        _emit(tc, nc, (xd, cw1td, cw2td, rwd, btd, od, identd), mybir, bass)
    nc.compile()
    _STATE["nc"] = nc
    return nc


def prep_in_maps(inputs):
    """Host-side marshalling: dtype casts + static layout transforms."""
    import ml_dtypes

    BF = ml_dtypes.bfloat16
    arr = {k: np.asarray(v, dtype=np.float32) for k, v in inputs.items()}
    x = np.ascontiguousarray(arr["x"].astype(BF))
    # block-diagonal control weights: one matmul emits both samples
    # stacked; coeff2 lands directly in the (s,oc) partition layout
    cw1t = np.zeros((128, 2 * HID), np.float32)
    cw1t[0:C, 0:HID] = arr["cw1"].T
    cw1t[C:128, HID:2 * HID] = arr["cw1"].T
    cw2t = np.zeros((2 * HID, E * 128), np.float32)
    for e in range(E):
        blk = arr["cw2"][e::E, :].T          # [HID, C] for expert e
        cw2t[0:HID, 128 * e:128 * e + C] = blk
        cw2t[HID:2 * HID, 128 * e + C:128 * (e + 1)] = blk
    shared = {"cw1t": np.ascontiguousarray(cw1t / (H * W)),
              "cw2t": np.ascontiguousarray(cw2t.astype(BF)),
              "ident": np.eye(128).astype(BF)}
    for l in range(3):
        w = arr[f"w{l+1}"].reshape(E, C, C, T9)      # [e, o, c, t]
        wt = w.transpose(0, 1, 3, 2).reshape(E, C, T9 * C)  # (t,c) free
        shared[f"rw{l+1}"] = np.ascontiguousarray(
            np.concatenate([wt, wt], axis=1).astype(BF))
        b = arr[f"b{l+1}"].T
        shared[f"bt{l+1}"] = np.ascontiguousarray(
            np.concatenate([b, b], axis=0))
    return [
        {"x": np.ascontiguousarray(x[S * i:S * (i + 1)]), **shared}
        for i in range(N_CORES)
    ]


def kernel(**inputs):
    from concourse.bass_utils import run_bass_kernel_spmd

    nc = _get_nc()
    in_maps = prep_in_maps(inputs)
    trace = bool(int(os.environ.get("KBENCH_TRACE", "0")))
    last_err = None
    for _attempt in range(3):
        try:
            res = run_bass_kernel_spmd(
                nc, in_maps, list(range(N_CORES)), trace=trace
            )
            _STATE["last"] = res
            return np.concatenate(
                [np.asarray(res.results[i]["out"]).astype(np.float32)
                 for i in range(N_CORES)], axis=0
            )
        except Exception as e:  # transient NRT device faults observed
            last_err = e
            if "UNRECOVERABLE" not in str(e) and "UNAVAILABLE" not in str(e):
                raise
    raise last_err
